# revision 18
# baseline (speedup 1.0000x reference)
"""Bidirectional Mamba block (BiT_MamSleep) on 8 TRN2 NeuronCores — v2.

Sharding: core c handles (batch b = c//2, direction dir = c%2); the pair
exchanges direction outputs with a pairwise bf16 AllReduce and computes the
small tail redundantly (identical SPMD program; direction is baked into the
per-core weights/masks).

v2 (vs the 697us baseline):
 - s-major selective scan: 32 tiles of (d-half h, state s), 128 d-rows on
   partitions, full L=2048 free dim. dt/dt*u stay resident in SBUF (no 16x
   partition replication); the small B/C rows are broadcast per tile from
   DRAM with 0-stride descriptors on two HWDGE queues.
 - state contraction over s = accumulating identity matmul (bf16, full PE
   rate) instead of per-tile mask matmuls.
 - all projection matmuls in bf16 (1 PE cycle/row vs 4 for fp32); only the
   tiny dt projection stays fp32 since scan decay exp(A*dt) is
   precision-critical, as are dt/dA (kept fp32).
 - all elementwise ops at full row length 2048 to amortize the ~450ns
   per-instruction DVE overhead; dbu/ycm multiplies alternate DVE/GpSimd.
"""
import sys

if '/opt/trn_rl_repo' not in sys.path:
    sys.path.insert(0, '/opt/trn_rl_repo')

import numpy as np
import ml_dtypes

import concourse.bass as bass
import concourse.bacc as bacc
import concourse.tile as tile
from concourse import mybir
from concourse.bass_utils import run_bass_kernel_spmd

ml_bf16 = np.float16

HID = 128
BATCH = 4
SEQ = 2048
D_STATE = 16
D_CONV = 4
D_INNER = 256
DT_RANK = 8

L = SEQ
C = HID
CW = 512           # matmul / PSUM chunk width
NCH = L // CW
f32 = mybir.dt.float32
bf16 = mybir.dt.float16   # "bf16" name kept; all 2-byte tiles are fp16
mult = mybir.AluOpType.mult
add = mybir.AluOpType.add
sub = mybir.AluOpType.subtract
AF = mybir.ActivationFunctionType

_PROGRAM = None


def _declare(nc):
    dp = lambda name, shape, dt=bf16: nc.declare_dram_parameter(
        name, list(shape), dt, isOutput=False)
    p = {}
    p['x'] = dp('x', (C, L), f32)
    for n in ('wlmT', 'wlgT', 'wcT', 'loT'):
        p[n] = dp(n, (C, C))
    p['inwT'] = dp('inwT', (C, 2 * D_INNER))
    p['inwcT'] = dp('inwcT', (C, 8 * 128))
    p['xpwT0'] = dp('xpwT0', (128, 80))
    p['xpwT1'] = dp('xpwT1', (128, 80))
    p['dtwT'] = dp('dtwT', (DT_RANK, D_INNER), f32)
    p['outwT0'] = dp('outwT0', (128, C))
    p['outwT1'] = dp('outwT1', (128, C))
    p['convw'] = dp('convw', (128, 2 * D_CONV), f32)
    p['avecs'] = dp('avecs', (128, 32), f32)      # col 16h+s = A[d-half h, s]
    p['ident16'] = dp('ident16', (128, 128))
    for n in ('conv_b', 'dt_b', 'dp_v'):
        p[n] = dp(n, (128, 2), f32)               # halves in columns
    for n in ('bias_lm', 'bias_lg', 'bias_c', 'lo_b', 'ln_g', 'ln_b',
              'm_fwd', 'm_bwd'):
        p[n] = dp(n, (C, 1), f32)
    p['y'] = nc.declare_dram_parameter('y', [C, L], f32, isOutput=True)
    return p


class B:
    """Builder state shared by the stage helpers."""


def _proj(b, lhsT, rhs, out, func, bias, out_cols=None, rows=C):
    """out[:, cs] = func(lhsT.T @ rhs[:, cs] + bias) per CW-chunk (PE + ACT)."""
    nc = b.nc
    for ci in range(NCH):
        cs = slice(ci * CW, (ci + 1) * CW)
        ocs = cs if out_cols is None else slice(out_cols + ci * CW,
                                                out_cols + (ci + 1) * CW)
        ps = b.ps.tile([rows, CW], f32, name='bank', tag='bank')
        nc.tensor.matmul(ps, lhsT, rhs[:, cs], start=True, stop=True)
        nc.scalar.activation(out[:, ocs], ps, func, bias=bias)


def _layernorm(b, x16, pref):
    """LayerNorm over the 128 channels per column: returns a bf16 gp tile
    holding (x - mean) * rsqrt(var + eps). Stats via ones-matmul (bf16); the
    mean/rstd rows are broadcast back across partitions with K=1 ones-row
    matmuls (PSUM)."""
    nc = b.nc
    rows = b.io.tile([128, L], bf16, name='lnrows', tag='lnrows')
    ex = rows[0:1, :]
    rr_ = ex  # same base-0 row reused per chunk (mean then rstd)
    nrm0 = b.gp_tile()
    sq2 = b.gp_tile()
    out = b.gp_tile()
    for ci in range(NCH):
        cs = slice(ci * CW, (ci + 1) * CW)
        ps0 = b.ps.tile([1, CW], f32, name='bank', tag='bank')
        nc.tensor.matmul(ps0, b.ones_col, x16[:, cs], start=True, stop=True)
        nc.scalar.activation(ex[:, cs], ps0, AF.Identity, bias=0.0, scale=1.0 / C)
        psb = b.ps.tile([128, CW], f32, name='bank', tag='bank')
        nc.tensor.matmul(psb, b.ones_row, ex[:, cs], start=True, stop=True)
        nc.vector.scalar_tensor_tensor(nrm0[:, cs], x16[:, cs], 1.0, psb, mult, sub)
        nc.scalar.activation(sq2[:, cs], nrm0[:, cs], AF.Square)
        psv = b.ps.tile([1, CW], f32, name='bank', tag='bank')
        nc.tensor.matmul(psv, b.ones_col, sq2[:, cs], start=True, stop=True)
        nc.scalar.activation(rr_[:, cs], psv, AF.Ln, bias=b.eps_t[:, :], scale=1.0 / C)
        nc.scalar.activation(rr_[:, cs], rr_[:, cs], AF.Exp, bias=0.0, scale=-0.5)
        psr = b.ps.tile([128, CW], f32, name='bank', tag='bank')
        nc.tensor.matmul(psr, b.ones_row, rr_[:, cs], start=True, stop=True)
        nc.vector.scalar_tensor_tensor(out[:, cs], nrm0[:, cs], 1.0, psr, mult, mult)
    return out


def _build_body(nc, tc, p, ctx):
    b = B()
    b.nc = nc
    b.io = ctx.enter_context(tc.tile_pool(name='io', bufs=1))
    b.gp = ctx.enter_context(tc.tile_pool(name='gp', bufs=4))
    b.rot = ctx.enter_context(tc.tile_pool(name='rot', bufs=3))
    b.ps = ctx.enter_context(tc.tile_pool(name='ps', bufs=3, space='PSUM'))
    b.py = ctx.enter_context(tc.tile_pool(name='py', bufs=1, space='PSUM'))
    b.dram = ctx.enter_context(tc.tile_pool(name='drm', bufs=1, space='DRAM'))
    b.gp_tile = lambda: b.gp.tile([C, L], bf16, name='g', tag='g')

    # ---- load weights/vectors ----
    W = {}
    for n, shape, dt in (('wlmT', (C, C), bf16), ('wlgT', (C, C), bf16),
                         ('wcT', (C, C), bf16), ('loT', (C, C), bf16),
                         ('inwT', (C, 2 * D_INNER), bf16),
                         ('inwcT', (C, 8 * 128), bf16),
                         ('xpwT0', (128, 80), bf16), ('xpwT1', (128, 80), bf16),
                         ('dtwT', (8, 256), f32),
                         ('outwT0', (128, C), bf16), ('outwT1', (128, C), bf16),
                         ('convw', (128, 8), f32), ('avecs', (128, 32), f32),
                         ('ident16', (128, 128), bf16)):
        W[n] = b.io.tile(list(shape), dt, name=n, tag=n)
        nc.sync.dma_start(out=W[n], in_=p[n][:, :])
    V = {}
    for n in ('conv_b', 'dt_b', 'dp_v'):
        V[n] = b.io.tile([128, 2], f32, name=n, tag=n)
        nc.sync.dma_start(out=V[n], in_=p[n][:, :])
    for n in ('bias_lm', 'bias_lg', 'bias_c', 'lo_b', 'ln_g', 'ln_b',
              'm_fwd', 'm_bwd'):
        V[n] = b.io.tile([C, 1], f32, name=n, tag=n)
        nc.sync.dma_start(out=V[n], in_=p[n][:, :])
    ones_col = b.io.tile([C, 1], bf16, name='ones_col', tag='ones_col')
    nc.vector.memset(ones_col, 1.0)
    b.ones_col = ones_col
    eps_t = b.io.tile([1, 1], f32, name='lneps', tag='lneps')
    nc.vector.memset(eps_t, 1e-5)
    b.eps_t = eps_t
    ones_row = b.io.tile([1, 128], bf16, name='ones_row', tag='ones_row')
    nc.vector.memset(ones_row, 1.0)
    b.ones_row = ones_row

    # x arrives f32; gpsimd (SWDGE) DMA casts to bf16 on the way in
    x16 = b.gp_tile()
    nc.gpsimd.dma_start(out=x16, in_=p['x'][:, :])

    # ---- P1: input layernorm over channels ----
    with nc.named_scope('P1_ln'):
        nrm = _layernorm(b, x16, 'l1')

    # ---- P2: projections ----
    with nc.named_scope('P2_proj'):
        xmf_pre = b.gp_tile()
        _proj(b, W['wlmT'], nrm, xmf_pre, AF.Identity, V['bias_lm'][:, :])
        gate = b.io.tile([C, L], bf16, name='gate', tag='gate')
        _proj(b, W['wlgT'], nrm, gate, AF.Silu, V['bias_lg'][:, :])
        xm = b.gp_tile()
        _proj(b, W['wcT'], xmf_pre, xm, AF.Silu, V['bias_c'][:, :])

        sz16 = []
        for h in range(2):
            szt = b.io.tile([128, L], bf16, name=f'sz{h}', tag=f'sz{h}')
            _proj(b, W['inwT'][:, 256 + 128 * h:256 + 128 * (h + 1)], xm, szt,
                  AF.Silu, 0.0)
            sz16.append(szt)

        # causal depthwise conv folded into the u in-projection: 4
        # accumulating matmuls with tap-scaled weights against a 3-padded xm.
        xm_pad = b.io.tile([128, D_CONV - 1 + L], bf16, name='xm_pad',
                           tag='xm_pad')
        nc.vector.memset(xm_pad[:, 0:D_CONV - 1], 0.0)
        nc.vector.tensor_copy(xm_pad[:, D_CONV - 1:], xm)
        uc = []
        for h in range(2):
            uct = b.io.tile([128, L], bf16, name=f'uc{h}', tag=f'uc{h}')
            for ci in range(NCH):
                cs = slice(ci * CW, (ci + 1) * CW)
                ps_u = b.ps.tile([128, CW], f32, name='bank', tag='bank')
                for kk in range(D_CONV):
                    nc.tensor.matmul(
                        ps_u, W['inwcT'][:, (4 * h + kk) * 128:(4 * h + kk + 1) * 128],
                        xm_pad[:, kk + ci * CW: kk + ci * CW + CW],
                        start=(kk == 0), stop=(kk == D_CONV - 1))
                nc.scalar.activation(uct[:, cs], ps_u, AF.Silu,
                                     bias=V['conv_b'][:, h:h + 1])
            uc.append(uct)

        # dbl = xp_w @ uc -> dtr(8 rows, f32), B(16), C(16); one 40-row matmul
        dtr = b.io.tile([8, L], f32, name='dtr', tag='dtr')
        b16 = b.io.tile([16, L], bf16, name='b16', tag='b16')
        c16 = b.io.tile([16, L], bf16, name='c16', tag='c16')
        for ci in range(NCH):
            cs = slice(ci * CW, (ci + 1) * CW)
            ps_dbl = b.ps.tile([80, CW], f32, name='bank', tag='bank')
            nc.tensor.matmul(ps_dbl, W['xpwT0'], uc[0][:, cs], start=True,
                             stop=False)
            nc.tensor.matmul(ps_dbl, W['xpwT1'], uc[1][:, cs], start=False,
                             stop=True)
            # rows padded to 32-partition-aligned groups: dtr 0:8, B 32:48, C 64:80
            nc.scalar.activation(dtr[:, cs], ps_dbl[0:8, :], AF.Identity, bias=0.0)
            nc.scalar.activation(b16[:, cs], ps_dbl[32:48, :], AF.Identity, bias=0.0)
            nc.scalar.activation(c16[:, cs], ps_dbl[64:80, :], AF.Identity, bias=0.0)

        # stage B/C rows in DRAM for 0-stride partition broadcasts in P3
        bd = b.dram.tile([16, L], bf16, name='bd', tag='bd')
        cd = b.dram.tile([16, L], bf16, name='cd', tag='cd')
        nc.sync.dma_start(out=bd, in_=b16)
        nc.sync.dma_start(out=cd, in_=c16)

        # dt = softplus(dt_w @ dtr + dt_b) = ln(1 + exp(z)); fp32 matmul —
        # the scan decay exp(A*dt) is precision-critical. dtu16 = dt*uc.
        dt_t = []
        dtu16 = []
        for h in range(2):
            dtt = b.io.tile([128, L], f32, name=f'dt{h}', tag=f'dt{h}')
            _proj(b, W['dtwT'][:, 128 * h:128 * (h + 1)], dtr, dtt, AF.Exp,
                  V['dt_b'][:, h:h + 1], rows=128)
            nc.scalar.activation(dtt, dtt, AF.Ln, bias=1.0, scale=1.0)
            dt_t.append(dtt)
            dtut = b.io.tile([128, L], bf16, name=f'dtu{h}', tag=f'dtu{h}')
            nc.vector.scalar_tensor_tensor(dtut, dtt, 1.0, uc[h], mult, mult)
            dtu16.append(dtut)

    # ---- P3: selective scan, s-major: tile (h, s) = 128 d-rows x L ----
    with nc.named_scope('P3_scan'):
        yz16 = []
        for h in range(2):
            psy = b.py.tile([128, L], f32, name='psy', tag='psy')
            for s in range(D_STATE):
                k = 16 * h + s
                # B[s]/C[s] rows broadcast to all 128 partitions (0-stride
                # DRAM read), on two different HWDGE queues.
                brep = b.rot.tile([128, L], bf16, name='brep', tag='brep', bufs=2)
                bsrc = bass.AP(tensor=bd.tensor, offset=bd.offset + s * L,
                               ap=[[0, 128], [1, L]])
                nc.sync.dma_start(out=brep, in_=bsrc)
                crep = b.rot.tile([128, L], bf16, name='crep', tag='crep', bufs=2)
                csrc = bass.AP(tensor=cd.tensor, offset=cd.offset + s * L,
                               ap=[[0, 128], [1, L]])
                nc.scalar.dma_start(out=crep, in_=csrc)

                # dbu = dtu * B[s]; ycm = ht * C[s]  (out-of-place f16 TT)
                dbu = b.rot.tile([128, L], bf16, name='dbu', tag='dbu', bufs=2)
                nc.vector.tensor_mul(dbu, dtu16[h], brep)

                da = b.rot.tile([128, L], bf16, name='da', tag='da', bufs=2)
                nc.scalar.activation(da, dt_t[h], AF.Exp, bias=0.0,
                                     scale=W['avecs'][:, k:k + 1])
                ht = b.rot.tile([128, L], bf16, name='ht', tag='ht')
                nc.vector.tensor_tensor_scan(ht, da, dbu, 0.0, mult, add)
                ycm = b.rot.tile([128, L], bf16, name='ycm', tag='ycm', bufs=2)
                nc.vector.tensor_mul(ycm, ht, crep)
                for ci in range(NCH):
                    ccs = slice(ci * CW, (ci + 1) * CW)
                    nc.tensor.matmul(psy[:, ccs], W['ident16'], ycm[:, ccs],
                                     start=(s == 0), stop=(s == D_STATE - 1),
                                     skip_group_check=True)
            # y1 = uc*Dp + psy ; yz = y1 * silu(z)
            tmp16 = b.rot.tile([128, L], bf16, name='tmp16', tag='tmp16', bufs=1)
            nc.vector.scalar_tensor_tensor(
                tmp16, uc[h], V['dp_v'][:, h:h + 1], psy, mult, add)
            yzt = b.io.tile([128, L], bf16, name=f'yz{h}', tag=f'yz{h}')
            nc.vector.scalar_tensor_tensor(yzt, tmp16, 1.0, sz16[h], mult, mult)
            yz16.append(yzt)

        # out projection: y_dir = out_w @ (y * silu(z))   (bf16 matmul)
        y16 = b.io.tile([C, L], bf16, name='y16', tag='y16')
        for ci in range(NCH):
            cs = slice(ci * CW, (ci + 1) * CW)
            ps_o = b.ps.tile([C, CW], f32, name='bank', tag='bank')
            nc.tensor.matmul(ps_o, W['outwT0'], yz16[0][:, cs], start=True,
                             stop=False)
            nc.tensor.matmul(ps_o, W['outwT1'], yz16[1][:, cs], start=False,
                             stop=True)
            nc.scalar.activation(y16[:, cs], ps_o, AF.Identity, bias=0.0)

    # ---- P4: flip (backward dir), select, pairwise exchange (bf16) ----
    with nc.named_scope('P4_exchange'):
        y_flip = b.io.tile([C, L], bf16, name='y_flip', tag='y_flip')
        nc.vector.tensor_copy(y_flip, y16[:, ::-1])
        y_sel = b.io.tile([C, L], bf16, name='y_sel', tag='y_sel')
        nc.vector.tensor_scalar_mul(y_sel, y16, V['m_fwd'][:, :])
        nc.vector.scalar_tensor_tensor(y_sel, y_flip, V['m_bwd'][:, :], y_sel,
                                       mult, add)

        cc_in = b.dram.tile([C, L], bf16, name='cc_in', tag='cc_in')
        cc_out = b.dram.tile([C, L], bf16, name='cc_out', tag='cc_out')
        nc.sync.dma_start(out=cc_in, in_=y_sel)
        nc.gpsimd.collective_compute(
            'AllReduce', add,
            replica_groups=[[0, 1], [2, 3], [4, 5], [6, 7]],
            ins=[cc_in.opt()], outs=[cc_out.opt()])
        y_sum = b.io.tile([C, L], bf16, name='y_sum', tag='y_sum')
        nc.sync.dma_start(out=y_sum, in_=cc_out)

    # ---- P5: tail: gate multiply, lo projection, final LN ----
    with nc.named_scope('P5_tail'):
        g1 = b.gp_tile()
        nc.vector.scalar_tensor_tensor(g1, y_sum, 1.0, gate, mult, mult)
        t2 = b.gp_tile()
        _proj(b, W['loT'], g1, t2, AF.Identity, V['lo_b'][:, :])

        o1 = _layernorm(b, t2, 'l2')
        out_sb = b.io.tile([C, L], f32, name='out_sb', tag='out_sb')
        nc.scalar.activation(out_sb, o1, AF.Identity, bias=V['ln_b'][:, :],
                             scale=V['ln_g'][:, :])
        nc.sync.dma_start(out=p['y'][:, :], in_=out_sb)


def _build_program():
    import contextlib
    nc = bacc.Bacc('TRN2', target_bir_lowering=False, debug=False,
                   num_devices=8, num_swdge_queues=2)
    p = _declare(nc)
    with tile.TileContext(nc) as tc:
        with contextlib.ExitStack() as ctx:
            _build_body(nc, tc, p, ctx)
    nc.compile()
    return nc


def _prep_core_inputs(inputs, bidx, d):
    g = lambda n: np.asarray(inputs[n], dtype=np.float32)
    x = g('x')
    ln_g = g('ln_g')
    ln_b = g('ln_b')
    pre = 'mf_' if d == 0 else 'mb_'
    P = lambda n: np.asarray(inputs[pre + n], dtype=np.float32)

    lm_w, lm_b = g('lm_w'), g('lm_b')
    lg_w, lg_b = g('lg_w'), g('lg_b')
    lo_w, lo_b = g('lo_w'), g('lo_b')
    if d == 0:
        wc, cb = g('cf_w'), g('cf_b')
    else:
        wc, cb = np.ascontiguousarray(g('cb_w')[:, ::-1]), g('cb_b')

    A = -np.exp(P('Alog'))                       # (256,16)
    # avecs[:, 16h+s] = A[128h:128h+128, s]
    avecs = np.concatenate([A[:128, :], A[128:, :]], axis=1).astype(np.float32)
    avecs = np.ascontiguousarray(avecs)

    halves = lambda v: np.ascontiguousarray(
        np.stack([v[:128], v[128:]], axis=1).astype(np.float32))
    col = lambda v: np.ascontiguousarray(v.astype(np.float32).reshape(-1, 1))
    T16 = lambda w: np.ascontiguousarray(w.T.astype(ml_bf16))

    xpwT40 = P('xp_w').T.astype(np.float32)                      # (256,40)
    xpwT = np.zeros((256, 80), np.float32)
    xpwT[:, 0:8] = xpwT40[:, 0:8]
    xpwT[:, 32:48] = xpwT40[:, 8:24]
    xpwT[:, 64:80] = xpwT40[:, 24:40]
    xpwT = np.ascontiguousarray(xpwT.astype(ml_bf16))
    outwT = np.ascontiguousarray(P('out_w').T.astype(ml_bf16))   # (256,128)
    cwn = P('conv_w')                            # (256,4)
    convw = np.ascontiguousarray(np.concatenate([cwn[:128], cwn[128:]], axis=1))

    inw = P('in_w').astype(np.float32)           # (512, 128)
    inwc = np.zeros((128, 8 * 128), np.float32)  # lhsT per (half,tap)
    for h in range(2):
        for kk in range(D_CONV):
            wblk = inw[128 * h:128 * (h + 1)] * cwn[128 * h:128 * (h + 1),
                                                    kk:kk + 1]
            inwc[:, (4 * h + kk) * 128:(4 * h + kk + 1) * 128] = wblk.T
    return {
        'x': np.ascontiguousarray(x[bidx]),
        'inwcT': np.ascontiguousarray(inwc.astype(ml_bf16)),
        'wlmT': T16(lm_w * ln_g[None, :]),
        'wlgT': T16(lg_w * ln_g[None, :]),
        'wcT': T16(wc),
        'loT': T16(lo_w),
        'inwT': T16(P('in_w')),
        'xpwT0': np.ascontiguousarray(xpwT[:128]),
        'xpwT1': np.ascontiguousarray(xpwT[128:]),
        'dtwT': np.ascontiguousarray(P('dt_w').T.astype(np.float32)),
        'outwT0': np.ascontiguousarray(outwT[:128]),
        'outwT1': np.ascontiguousarray(outwT[128:]),
        'convw': convw,
        'avecs': avecs,
        'ident16': np.eye(128, dtype=ml_bf16),
        'conv_b': halves(P('conv_b')),
        'dt_b': halves(P('dt_b')),
        'dp_v': halves(P('D')),
        'bias_lm': col(lm_w @ ln_b + lm_b),
        'bias_lg': col(lg_w @ ln_b + lg_b),
        'bias_c': col(cb),
        'lo_b': col(lo_b),
        'ln_g': col(ln_g),
        'ln_b': col(ln_b),
        'm_fwd': np.full((C, 1), 1.0 if d == 0 else 0.0, np.float32),
        'm_bwd': np.full((C, 1), 0.0 if d == 0 else 1.0, np.float32),
    }


def get_program():
    global _PROGRAM
    if _PROGRAM is None:
        _PROGRAM = _build_program()
    return _PROGRAM


def run(inputs, **run_kwargs):
    nc = get_program()
    in_maps = [_prep_core_inputs(inputs, c // 2, c % 2) for c in range(8)]
    res = run_bass_kernel_spmd(nc, in_maps, core_ids=list(range(8)), **run_kwargs)
    out = np.stack([res.results[2 * b]['y'] for b in range(BATCH)], axis=0)
    return out, res


def kernel(**inputs) -> np.ndarray:
    out, _ = run(inputs)
    return out.astype(np.float32)
